# revision 32
# baseline (speedup 1.0000x reference)
"""MoE FFN (sparse expert-parallel) Trainium2 kernel.

Strategy: expert-parallel across 8 NeuronCores. Core e holds expert e's
FFN weights and computes, for ALL 8192 tokens: the gate (fp32, on device),
then routes its expert's <=CAP selected tokens through the FFN (bf16
matmuls, fp32 accumulate), scaling by the top-2-renormalized gate weight.
Host sums the 8 partial outputs (the "psum" combine of the unshard step).

The per-core expert selection is encoded purely in the input layout: each
core receives Wg/bg with expert columns permuted so its own expert is
column 0 — the gate math is permutation-equivariant, so column 0 of the
weight matrix is always "my expert".

v2 compaction: the slot table (slot -> token id, gate weight) is built
entirely in SBUF with one-hot permutation matmuls instead of per-token
indirect DRAM scatters (8192 8-byte HBM writes cost ~170ns each in SDMA
descriptor drain — ~1.4ms — which dominated v1). For each 128-slot tile
st, only the few token tiles g whose slot ranges can intersect it
contribute; that block list is computed host-side from the routing (like
CAP already is) as a union over all 8 experts plus safety margin, so the
NEFF structure stays expert-independent (SPMD).
"""
import sys

sys.path.insert(0, "/opt/trn_rl_repo")

import numpy as np
import ml_dtypes

import bass_rust
import concourse.bass as bass
import concourse.mybir as mybir
import concourse.bass_utils as bu
from concourse.tile import TileContext

BF16 = ml_dtypes.bfloat16

B, T, C, E, H = 4, 2048, 1024, 8, 4096
NT = B * T          # 8192 tokens
P = 128
KC = C // P         # 8 k-tiles over C
KH = H // P         # 32 k-tiles over H
NG = NT // P        # 64 token tiles
GCH = 256           # gate chunk tokens (2 tiles)
NGCH = NT // GCH    # 32 gate chunks
CAP = 2176          # slot capacity (seed-0 max expert count is 2115)
NSL = CAP // P      # 17 slot tiles
FFN_CHUNKS = [(0, 4), (4, 4), (8, 4), (12, 4), (16, 1)]  # (st0, n slot tiles)
CCH = C // 512      # 2 output column chunks
XR_ROWS = NT + P    # x rows + zero row at NT (pad slots gather/scatter there)
BIG = 9999.0        # "unselected" slot id sentinel

F32 = mybir.dt.float32
BF = mybir.dt.bfloat16
I32 = mybir.dt.int32
Relu = mybir.ActivationFunctionType.Relu
Exp = mybir.ActivationFunctionType.Exp
Ident = mybir.ActivationFunctionType.Identity
Alu = mybir.AluOpType


def _split_excess_waits(nc):
    """walrus codegen allows 1 sem-wait per instruction (2 on
    EventSemaphore). Move excess waits onto same-engine EventSemaphore
    insts placed just before (engine program order preserves semantics)."""
    for f in nc.m.functions:
        for bb in f.blocks:
            new = []
            changed = False
            for inst in bb.instructions:
                si = inst.sync_info
                cap = 2 if isinstance(inst, mybir.InstEventSemaphore) else 1
                if si is not None and len(si.on_wait) > cap:
                    waits = list(si.on_wait)
                    extra, keep = waits[:-cap], waits[-cap:]
                    for i in range(0, len(extra), 2):
                        w = mybir.InstEventSemaphore(
                            name=f"{inst.name}_presem{i}", ins=[], outs=[])
                        w.engine = inst.engine
                        w.sync_info = bass_rust.SyncInfo(
                            on_wait=extra[i:i + 2], on_update=[])
                        new.append(w)
                        changed = True
                    inst.sync_info = bass_rust.SyncInfo(
                        on_wait=keep, on_update=list(si.on_update))
                new.append(inst)
            if changed:
                bb.instructions = new


def _strip_scatter_waw(nc):
    """The 20 per-slot-tile output scatters (Pool-engine DMAs with a
    dynamic out AP) get WAW-chained by Tile at whole-tensor granularity,
    each waiting the previous one's DMA-completion semaphore. Their
    writes are row-disjoint (collisions only on the trash row, whose
    value is never read), and nothing reads `out` mid-kernel, so drop
    those DMASW waits. The kernel-tail drain still covers completion."""
    n_strip = 0
    for f in nc.m.functions:
        for bb in f.blocks:
            for inst in bb.instructions:
                if (not isinstance(inst, mybir.InstDMACopy)
                        or inst.engine != mybir.EngineType.Pool
                        or not inst.outs):
                    continue
                if getattr(inst.outs[0], "dynamic_ap_info", None) is None:
                    continue
                si = inst.sync_info
                if si is None or not si.on_wait:
                    continue
                keep = [w for w in si.on_wait
                        if not str(w.ant_name).startswith("DMASW")]
                if len(keep) != len(si.on_wait):
                    n_strip += len(si.on_wait) - len(keep)
                    inst.sync_info = bass_rust.SyncInfo(
                        on_wait=keep, on_update=list(si.on_update))
    return n_strip


def _build_dense():
    nc = bass.Bass()
    xt = nc.declare_dram_parameter("xt", [C, NT], F32, isOutput=False)
    xtb = nc.declare_dram_parameter("xtb", [C, NT], BF, isOutput=False)
    w1 = nc.declare_dram_parameter("w1", [C, H], BF, isOutput=False)
    b1c = nc.declare_dram_parameter("b1c", [P, KH], F32, isOutput=False)
    w2 = nc.declare_dram_parameter("w2", [H, C], BF, isOutput=False)
    b2r = nc.declare_dram_parameter("b2r", [1, C], BF, isOutput=False)
    wgp = nc.declare_dram_parameter("wgp", [C, E], F32, isOutput=False)
    bgp = nc.declare_dram_parameter("bgp", [1, E], F32, isOutput=False)
    out = nc.declare_dram_parameter("out", [NT, C], F32, isOutput=True)
    TOKCH = 512
    NCH = NT // TOKCH

    with TileContext(nc) as tc:
        with tc.tile_pool(name="wpool", bufs=1) as wpool, \
             tc.tile_pool(name="gpool", bufs=4) as gpool, \
             tc.tile_pool(name="xgpool", bufs=2) as xgpool, \
             tc.tile_pool(name="xbpool", bufs=1) as xbpool, \
             tc.tile_pool(name="htpool", bufs=1) as htpool, \
             tc.tile_pool(name="ypool", bufs=3) as ypool, \
             tc.tile_pool(name="psg", bufs=2, space="PSUM") as psgp, \
             tc.tile_pool(name="ps1", bufs=2, space="PSUM") as ps1p, \
             tc.tile_pool(name="ps2", bufs=2, space="PSUM") as ps2p:

            # ---- resident weights / constants
            w1_sb = []
            for k in range(KC):
                t = wpool.tile([P, H], BF, tag=f"w1k{k}")
                nc.sync.dma_start(out=t[:], in_=w1[k * P:(k + 1) * P, :])
                w1_sb.append(t)
            w2_sb = []
            for h in range(KH):
                t = wpool.tile([P, C], BF, tag=f"w2k{h}")
                nc.sync.dma_start(out=t[:], in_=w2[h * P:(h + 1) * P, :])
                w2_sb.append(t)
            wg_sb = []
            for k in range(KC):
                t = wpool.tile([P, E], F32, tag=f"wgk{k}")
                nc.sync.dma_start(out=t[:], in_=wgp[k * P:(k + 1) * P, :])
                wg_sb.append(t)
            b1c_sb = wpool.tile([P, KH], F32, tag="b1c")
            nc.sync.dma_start(out=b1c_sb[:], in_=b1c[:])
            b2r_sb = wpool.tile([1, C], BF, tag="b2r")
            nc.sync.dma_start(out=b2r_sb[:], in_=b2r[:])
            bg_sb = wpool.tile([1, E], F32, tag="bgp")
            nc.sync.dma_start(out=bg_sb[:], in_=bgp[:])
            ones_f = wpool.tile([1, P], F32, tag="ones_f")
            nc.vector.memset(ones_f[:], 1.0)
            ones_b = wpool.tile([1, P], BF, tag="ones_b")
            nc.vector.memset(ones_b[:], 1.0)
            # per-token gate weight of "my" expert, column g = token tile g
            wcol = wpool.tile([P, NG], F32, tag="wcol")

            # ---- gate phase: fp32 logits -> softmax -> top2 renorm weight
            for g in range(NG):
                xg = [xgpool.tile([P, P], F32, tag=f"xg{k}", name=f"xg{k}")
                      for k in range(KC)]
                for k in range(KC):
                    nc.sync.dma_start(
                        out=xg[k][:],
                        in_=xt[k * P:(k + 1) * P, g * P:(g + 1) * P])
                psg = psgp.tile([P, E], F32)
                for k in range(KC):
                    nc.tensor.matmul(out=psg[:], lhsT=xg[k][:], rhs=wg_sb[k][:],
                                     start=(k == 0), stop=False)
                nc.tensor.matmul(out=psg[:], lhsT=ones_f[:], rhs=bg_sb[:],
                                 start=False, stop=True)
                m = gpool.tile([P, 1], F32, tag="gm")
                nc.vector.reduce_max(out=m[:], in_=psg[:],
                                     axis=mybir.AxisListType.X)
                nm = gpool.tile([P, 1], F32, tag="gnm")
                nc.vector.tensor_scalar_mul(nm[:], m[:], -1.0)
                pexp = gpool.tile([P, E], F32, tag="gpexp")
                nc.scalar.activation(pexp[:], psg[:], Exp, bias=nm[:])
                s = gpool.tile([P, 1], F32, tag="gs")
                nc.vector.reduce_sum(out=s[:], in_=pexp[:],
                                     axis=mybir.AxisListType.X)
                rs = gpool.tile([P, 1], F32, tag="grs")
                nc.vector.reciprocal(rs[:], s[:])
                pn = gpool.tile([P, E], F32, tag="gpn")
                nc.vector.tensor_scalar_mul(pn[:], pexp[:], rs[:])
                top8 = gpool.tile([P, E], F32, tag="gtop8")
                nc.vector.max(out=top8[:], in_=pn[:])
                etop = gpool.tile([P, 2], F32, tag="getop")
                nc.scalar.activation(etop[:], top8[:, 0:2], Exp)
                d = gpool.tile([P, 1], F32, tag="gd")
                nc.vector.reduce_sum(out=d[:], in_=etop[:],
                                     axis=mybir.AxisListType.X)
                rd = gpool.tile([P, 1], F32, tag="grd")
                nc.vector.reciprocal(rd[:], d[:])
                ep0 = gpool.tile([P, 1], F32, tag="gep0")
                nc.scalar.activation(ep0[:], pn[:, 0:1], Exp)
                mask0 = gpool.tile([P, 1], F32, tag="gmask0")
                nc.vector.tensor_tensor(out=mask0[:], in0=pn[:, 0:1],
                                        in1=top8[:, 1:2],
                                        op=Alu.is_ge)
                t1 = gpool.tile([P, 1], F32, tag="gt1")
                nc.vector.tensor_tensor(out=t1[:], in0=ep0[:], in1=mask0[:],
                                        op=Alu.mult)
                nc.vector.tensor_tensor(out=wcol[:, g:g + 1], in0=t1[:],
                                        in1=rd[:], op=Alu.mult)

            # ---- FFN phase
            for q in range(NCH):
                xb = [xbpool.tile([P, TOKCH], BF, tag=f"xb{k}", name=f"xb{k}")
                      for k in range(KC)]
                for k in range(KC):
                    nc.sync.dma_start(
                        out=xb[k][:],
                        in_=xtb[k * P:(k + 1) * P,
                                q * TOKCH:(q + 1) * TOKCH])
                ht = []
                for h in range(KH):
                    ps1 = ps1p.tile([P, TOKCH], F32)
                    for k in range(KC):
                        nc.tensor.matmul(
                            out=ps1[:],
                            lhsT=w1_sb[k][:, h * P:(h + 1) * P],
                            rhs=xb[k][:],
                            start=(k == 0), stop=(k == KC - 1))
                    htt = htpool.tile([P, TOKCH], BF, tag=f"ht{h}")
                    nc.scalar.activation(htt[:], ps1[:], Relu,
                                         bias=b1c_sb[:, h:h + 1])
                    ht.append(htt)
                for tt in range(TOKCH // P):
                    g = q * (TOKCH // P) + tt
                    for cc in range(CCH):
                        ps2 = ps2p.tile([P, 512], F32)
                        for h in range(KH):
                            nc.tensor.matmul(
                                out=ps2[:],
                                lhsT=ht[h][:, tt * P:(tt + 1) * P],
                                rhs=w2_sb[h][:, cc * 512:(cc + 1) * 512],
                                start=(h == 0), stop=False)
                        nc.tensor.matmul(
                            out=ps2[:], lhsT=ones_b[:],
                            rhs=b2r_sb[:, cc * 512:(cc + 1) * 512],
                            start=False, stop=True)
                        y = ypool.tile([P, 512], F32, tag="y")
                        nc.vector.tensor_scalar_mul(y[:], ps2[:],
                                                    wcol[:, g:g + 1])
                        nc.sync.dma_start(
                            out=out[g * P:(g + 1) * P,
                                    cc * 512:(cc + 1) * 512],
                            in_=y[:])

    _split_excess_waits(nc)
    return nc


def _build_sparse(blocks):
    """Sparse expert-parallel kernel. `blocks[st]` lists the token tiles
    that may contribute slots to slot tile st (host-computed union over
    experts, with margin)."""
    nc = bass.Bass()
    xt = nc.declare_dram_parameter("xt", [C, NT], F32, isOutput=False)
    xrb = nc.declare_dram_parameter("xrb", [XR_ROWS, C], BF, isOutput=False)
    w1 = nc.declare_dram_parameter("w1", [C, H], BF, isOutput=False)
    b1c = nc.declare_dram_parameter("b1c", [P, KH], F32, isOutput=False)
    w2 = nc.declare_dram_parameter("w2", [H, C], BF, isOutput=False)
    b2r = nc.declare_dram_parameter("b2r", [1, C], BF, isOutput=False)
    wgp = nc.declare_dram_parameter("wgp", [C, E], F32, isOutput=False)
    bgc = nc.declare_dram_parameter("bgc", [E, 1], F32, isOutput=False)
    out = nc.declare_dram_parameter("out", [XR_ROWS, C], F32, isOutput=True)

    with TileContext(nc) as tc:
        with tc.tile_pool(name="wpool", bufs=1) as wpool, \
             tc.tile_pool(name="gpool", bufs=2) as gpool, \
             tc.tile_pool(name="xgpool", bufs=1) as xgpool, \
             tc.tile_pool(name="ptpool", bufs=3) as ptpool, \
             tc.tile_pool(name="selpool", bufs=1) as selpool, \
             tc.tile_pool(name="xrpool", bufs=2) as xrpool, \
             tc.tile_pool(name="xtspool", bufs=1) as xtspool, \
             tc.tile_pool(name="htpool", bufs=1) as htpool, \
             tc.tile_pool(name="ypool", bufs=2) as ypool, \
             tc.tile_pool(name="psgp", bufs=2, space="PSUM") as psgp, \
             tc.tile_pool(name="pcp", bufs=2, space="PSUM") as pcp, \
             tc.tile_pool(name="ps1", bufs=2, space="PSUM") as ps1p, \
             tc.tile_pool(name="ps2", bufs=2, space="PSUM") as ps2p:

            # ---- gate weights / constants (FFN weights load later so the
            # gate's x streaming isn't queued behind 17MB of W1/W2)
            wg_sb = []
            for k in range(KC):
                t = wpool.tile([P, E], F32, tag=f"wgk{k}", name=f"wgk{k}")
                nc.sync.dma_start(out=t[:], in_=wgp[k * P:(k + 1) * P, :])
                wg_sb.append(t)
            bgc_sb = wpool.tile([E, 1], F32, tag="bgc")
            nc.sync.dma_start(out=bgc_sb[:], in_=bgc[:])
            ones_f = wpool.tile([1, P], F32, tag="ones_f")
            nc.vector.memset(ones_f[:], 1.0)
            ones_c = wpool.tile([P, 1], F32, tag="ones_c")
            nc.vector.memset(ones_c[:], 1.0)
            ones_b = wpool.tile([1, P], BF, tag="ones_b")
            nc.vector.memset(ones_b[:], 1.0)
            iotac = wpool.tile([P, 1], F32, tag="iotac")
            nc.gpsimd.iota(iotac[:], pattern=[[0, 1]], base=0,
                           channel_multiplier=1,
                           allow_small_or_imprecise_dtypes=True)
            iota2d = wpool.tile([P, P], F32, tag="iota2d")
            nc.gpsimd.iota(iota2d[:], pattern=[[1, P]], base=0,
                           channel_multiplier=0,
                           allow_small_or_imprecise_dtypes=True)
            # uts[p, s] = 1 if s > p  (strict upper triangle)
            uts_sb = wpool.tile([P, P], F32, tag="uts")
            nc.vector.tensor_scalar(out=uts_sb[:], in0=iota2d[:],
                                    scalar1=iotac[:, 0:1], scalar2=None,
                                    op0=Alu.is_gt)
            # bf16 identity for PE transposes, f32 identity for the gate
            idn_bf = wpool.tile([P, P], BF, tag="idn_bf")
            nc.vector.tensor_scalar(out=idn_bf[:], in0=iota2d[:],
                                    scalar1=iotac[:, 0:1], scalar2=None,
                                    op0=Alu.is_equal)
            idn_f = wpool.tile([E, E], F32, tag="idn_f")
            nc.vector.tensor_scalar(out=idn_f[:], in0=iota2d[0:E, 0:E],
                                    scalar1=iotac[0:E, 0:1], scalar2=None,
                                    op0=Alu.is_equal)
            # bf16 free-dim iota for the compaction one-hot compares
            # (values 0..127 are bf16-exact)
            iota2b = wpool.tile([P, P], BF, tag="iota2b")
            nc.vector.tensor_copy(out=iota2b[:], in_=iota2d[:])
            # per-token-tile matmul rhs blocks [pcol | 128g | w | 1],
            # one tile per 32-tile half so wave-A compaction can depend on
            # the first half of the gate only
            HG = NG // 2
            rhs_half, mcol_half = [], []
            for hb in range(2):
                rh = wpool.tile([P, 4 * HG], BF, tag=f"rhs{hb}",
                                name=f"rhs{hb}")
                rv = rh[:].rearrange("p (g c) -> p c g", c=4)
                nc.gpsimd.iota(rv[:, 0, :], pattern=[[0, HG]], base=0,
                               channel_multiplier=1,
                               allow_small_or_imprecise_dtypes=True)
                nc.gpsimd.iota(rv[:, 1, :], pattern=[[P, HG]],
                               base=P * HG * hb, channel_multiplier=0,
                               allow_small_or_imprecise_dtypes=True)
                nc.vector.memset(rv[:, 3, :], 1.0)
                rhs_half.append(rh)
                mcol_half.append(wpool.tile([P, HG], F32, tag=f"mcol{hb}",
                                              name=f"mcol{hb}"))

            # ---- gate phase: logits in [E, tok] orientation (stationary
            # Wg is only 8 columns so the fp32 matmul streams the x chunk;
            # the [tok, E] orientation pays a huge fp32 LDWEIGHTS per token
            # tile instead). PE-transpose each 128-token tile to [tok, E]
            # for the softmax. No max-subtraction: |logits| < 3 here.
            GT = 512

            def gate_chunk(q):
                hb, gofs = divmod(q * (GT // P), HG)
                mcol = mcol_half[hb]
                rhs_all = rhs_half[hb]
                xtc = [xgpool.tile([P, GT], F32, tag=f"xtc{k}",
                                   name=f"xtc{k}") for k in range(KC)]
                for k in range(KC):
                    nc.sync.dma_start(
                        out=xtc[k][:],
                        in_=xt[k * P:(k + 1) * P, q * GT:(q + 1) * GT])
                psgT = pcp.tile([E, GT], F32, tag="pc", name="psgT")
                for k in range(KC):
                    nc.tensor.matmul(out=psgT[:], lhsT=wg_sb[k][:],
                                     rhs=xtc[k][:],
                                     start=(k == 0), stop=(k == KC - 1))
                gT = gpool.tile([E, GT], F32, tag="gT", name="gT", bufs=1)
                nc.vector.tensor_scalar_add(gT[:], psgT[:], bgc_sb[:, 0:1])
                for i in range(GT // P):
                    g = gofs + i
                    psg = psgp.tile([P, E], F32, tag="psg", name="psg")
                    nc.tensor.transpose(out=psg[:],
                                        in_=gT[:, i * P:(i + 1) * P],
                                        identity=idn_f[:])
                    pexp = gpool.tile([P, E], F32, tag="gpexp", name="gpexp")
                    nc.scalar.activation(pexp[:], psg[:], Exp)
                    s = gpool.tile([P, 1], F32, tag="gs", name="gs")
                    nc.vector.reduce_sum(out=s[:], in_=pexp[:],
                                         axis=mybir.AxisListType.X)
                    rs = gpool.tile([P, 1], F32, tag="grs", name="grs")
                    nc.vector.reciprocal(rs[:], s[:])
                    pn = gpool.tile([P, E], F32, tag="gpn", name="gpn")
                    nc.vector.tensor_scalar_mul(pn[:], pexp[:], rs[:])
                    top8 = gpool.tile([P, E], F32, tag="gtop8", name="gtop8")
                    nc.vector.max(out=top8[:], in_=pn[:])
                    etop = gpool.tile([P, 2], F32, tag="getop", name="getop")
                    nc.scalar.activation(etop[:], top8[:, 0:2], Exp)
                    d = gpool.tile([P, 1], F32, tag="gd", name="gd")
                    nc.vector.reduce_sum(out=d[:], in_=etop[:],
                                         axis=mybir.AxisListType.X)
                    rd = gpool.tile([P, 1], F32, tag="grd", name="grd")
                    nc.vector.reciprocal(rd[:], d[:])
                    ep0 = gpool.tile([P, 1], F32, tag="gep0", name="gep0")
                    nc.scalar.activation(ep0[:], pn[:, 0:1], Exp)
                    nc.vector.tensor_tensor(out=mcol[:, g:g + 1],
                                            in0=pn[:, 0:1],
                                            in1=top8[:, 1:2],
                                            op=Alu.is_ge)
                    # w = ep0 * mask * rd  (both ops mult -> grouping-safe)
                    nc.vector.tensor_scalar(
                        out=rhs_all[:, 4 * g + 2:4 * g + 3], in0=ep0[:],
                        scalar1=mcol[:, g:g + 1], scalar2=rd[:, 0:1],
                        op0=Alu.mult, op1=Alu.mult)

            posm_half = [None, None]
            tot_a = gpool.tile([1, 1], F32, tag="tot_a", bufs=1)

            def positions_half(hb):
                """posm for token tiles [hb*HG, (hb+1)*HG); depends only on
                gate halves <= hb."""
                mcol = mcol_half[hb]
                ps_cnt = pcp.tile([HG, 1], F32, tag="pc", name=f"ps_cnt{hb}")
                nc.tensor.matmul(out=ps_cnt[:], lhsT=mcol[:], rhs=ones_c[:],
                                 start=True, stop=True)
                cnt_sb = gpool.tile([HG, 1], F32, tag="cnt_sb")
                nc.vector.tensor_copy(out=cnt_sb[:], in_=ps_cnt[:])
                if hb == 0:
                    ps_tot = pcp.tile([1, 1], F32, tag="pc", name="ps_tot")
                    nc.tensor.matmul(out=ps_tot[:], lhsT=cnt_sb[:],
                                     rhs=ones_c[0:HG, 0:1],
                                     start=True, stop=True)
                    nc.vector.tensor_copy(out=tot_a[:], in_=ps_tot[:])
                ps_brow = pcp.tile([1, HG], F32, tag="pc", name=f"ps_brow{hb}")
                nc.tensor.matmul(out=ps_brow[:], lhsT=cnt_sb[:],
                                 rhs=uts_sb[0:HG, 0:HG],
                                 start=True, stop=True)
                brow_sb = gpool.tile([1, HG], F32, tag="brow_sb")
                if hb == 0:
                    nc.vector.tensor_copy(out=brow_sb[:], in_=ps_brow[:])
                else:
                    nc.vector.tensor_scalar(
                        out=brow_sb[:], in0=ps_brow[:],
                        scalar1=tot_a[0:1, 0:1], scalar2=None, op0=Alu.add)
                ps_pos = pcp.tile([P, HG], F32, tag="pc", name=f"ps_pos{hb}")
                nc.tensor.matmul(out=ps_pos[:], lhsT=uts_sb[:], rhs=mcol[:],
                                 start=True, stop=False)
                nc.tensor.matmul(out=ps_pos[:], lhsT=ones_f[:],
                                 rhs=brow_sb[:], start=False, stop=True)
                pos_t = gpool.tile([P, HG], F32, tag="pos_t")
                nc.vector.tensor_scalar_add(pos_t[:], ps_pos[:], -BIG)
                pos_m = gpool.tile([P, HG], F32, tag="pos_m")
                nc.vector.tensor_tensor(out=pos_m[:], in0=pos_t[:],
                                        in1=mcol[:], op=Alu.mult)
                posm = gpool.tile([P, HG], F32, tag=f"posm{hb}", bufs=1)
                nc.vector.tensor_scalar_add(posm[:], pos_m[:], BIG)
                posm_half[hb] = posm

            id_tiles, w_tiles = {}, {}

            def compact_st(st):
                """one-hot matmuls gather (token id, weight) into SBUF
                slot order for slot tile st."""
                blks = blocks[st]
                pc = pcp.tile([P, 4], F32, tag="pc", name=f"pc{st}")
                for bi, g in enumerate(blks):
                    hb, gl = divmod(g, HG)
                    # pt[t, s] = (s + 128*st == posm[t]), one fused DVE op
                    pt = ptpool.tile([P, P], BF, tag="pt", name="pt")
                    nc.vector.tensor_scalar(
                        out=pt[:], in0=iota2b[:],
                        scalar1=float(P * st),
                        scalar2=posm_half[hb][:, gl:gl + 1],
                        op0=Alu.add, op1=Alu.is_equal)
                    nc.tensor.matmul(
                        out=pc[:], lhsT=pt[:],
                        rhs=rhs_half[hb][:, 4 * gl:4 * gl + 4],
                        start=(bi == 0), stop=(bi == len(blks) - 1))
                pcsb = gpool.tile([P, 4], F32, tag="pcsb", name="pcsb")
                nc.vector.tensor_copy(out=pcsb[:], in_=pc[:])
                u = gpool.tile([P, 1], F32, tag="selu", name="selu")
                nc.vector.tensor_tensor(out=u[:], in0=pcsb[:, 0:1],
                                        in1=pcsb[:, 1:2], op=Alu.add)
                t8 = gpool.tile([P, 1], F32, tag="selt8", name="selt8")
                nc.vector.tensor_scalar_add(t8[:], u[:], float(NT))
                # idf = id + NT*(1-valid): pad slots -> trash row NT
                idf = gpool.tile([P, 1], F32, tag="selidf", name="selidf")
                nc.scalar.activation(idf[:], pcsb[:, 3:4], Ident,
                                     bias=t8[:, 0:1], scale=float(-NT))
                idi = selpool.tile([P, 1], I32, tag=f"idi{st}",
                                   name=f"idi{st}")
                nc.vector.tensor_copy(out=idi[:], in_=idf[:])
                wst = selpool.tile([P, 1], F32, tag=f"wst{st}",
                                   name=f"wst{st}")
                nc.vector.tensor_copy(out=wst[:], in_=pcsb[:, 2:3])
                id_tiles[st] = idi
                w_tiles[st] = wst

            wave_a = [st for st in range(NSL) if max(blocks[st]) < HG]
            wave_b = [st for st in range(NSL) if st not in wave_a]

            # gate first half, then wave-A routing/compaction (overlaps the
            # gate's second half), then the rest
            for q in range(NT // GT // 2):
                gate_chunk(q)
            positions_half(0)
            for st in wave_a:
                compact_st(st)
            for q in range(NT // GT // 2, NT // GT):
                gate_chunk(q)

            # ---- FFN weights (streamed during the gate phase)
            w1_sb = []
            for k in range(KC):
                t = wpool.tile([P, H], BF, tag=f"w1k{k}", name=f"w1k{k}")
                nc.sync.dma_start(out=t[:], in_=w1[k * P:(k + 1) * P, :])
                w1_sb.append(t)
            w2_sb = []
            for h in range(KH):
                t = wpool.tile([P, C], BF, tag=f"w2k{h}", name=f"w2k{h}")
                nc.sync.dma_start(out=t[:], in_=w2[h * P:(h + 1) * P, :])
                w2_sb.append(t)
            b1c_sb = wpool.tile([P, KH], F32, tag="b1c")
            nc.sync.dma_start(out=b1c_sb[:], in_=b1c[:])
            b2r_sb = wpool.tile([1, C], BF, tag="b2r")
            nc.sync.dma_start(out=b2r_sb[:], in_=b2r[:])

            positions_half(1)
            for st in wave_b:
                compact_st(st)

            # ---- FFN over CAP slots
            for st0, nst in FFN_CHUNKS:
                TOK = nst * P
                xts = [xtspool.tile([P, 512], BF, tag=f"xts{k}",
                                    name=f"xts{k}") for k in range(KC)]
                for i in range(nst):
                    st = st0 + i
                    xrow = xrpool.tile([P, C], BF, tag="xrow", name="xrow")
                    nc.gpsimd.indirect_dma_start(
                        out=xrow[:], out_offset=None, in_=xrb[:],
                        in_offset=bass.IndirectOffsetOnAxis(
                            ap=id_tiles[st][:, :1], axis=0),
                        bounds_check=XR_ROWS - 1, oob_is_err=False)
                    for ck in range(KC):
                        pstt = psgp.tile([P, P], BF, tag="psg", name="pstt")
                        nc.tensor.transpose(
                            out=pstt[:], in_=xrow[:, ck * P:(ck + 1) * P],
                            identity=idn_bf[:])
                        nc.vector.tensor_copy(
                            out=xts[ck][:, i * P:(i + 1) * P], in_=pstt[:])
                ht = []
                for h in range(KH):
                    ps1 = ps1p.tile([P, 512], F32, tag="ps1", name="ps1")
                    for k in range(KC):
                        nc.tensor.matmul(
                            out=ps1[:, 0:TOK],
                            lhsT=w1_sb[k][:, h * P:(h + 1) * P],
                            rhs=xts[k][:, 0:TOK],
                            start=(k == 0), stop=(k == KC - 1))
                    htt = htpool.tile([P, 512], BF, tag=f"ht{h}",
                                      name=f"ht{h}")
                    nc.scalar.activation(htt[:, 0:TOK], ps1[:, 0:TOK], Relu,
                                         bias=b1c_sb[:, h:h + 1])
                    ht.append(htt)
                for i in range(nst):
                    st = st0 + i
                    y = ypool.tile([P, C], F32, tag="y", name="y")
                    for cc in range(CCH):
                        ps2 = ps2p.tile([P, 512], F32, tag="ps2", name="ps2")
                        for h in range(KH):
                            nc.tensor.matmul(
                                out=ps2[:],
                                lhsT=ht[h][:, i * P:(i + 1) * P],
                                rhs=w2_sb[h][:, cc * 512:(cc + 1) * 512],
                                start=(h == 0), stop=False)
                        nc.tensor.matmul(
                            out=ps2[:], lhsT=ones_b[:],
                            rhs=b2r_sb[:, cc * 512:(cc + 1) * 512],
                            start=False, stop=True)
                        nc.vector.tensor_scalar_mul(
                            y[:, cc * 512:(cc + 1) * 512], ps2[:],
                            w_tiles[st][:])
                    nc.gpsimd.indirect_dma_start(
                        out=out[:],
                        out_offset=bass.IndirectOffsetOnAxis(
                            ap=id_tiles[st][:, :1], axis=0),
                        in_=y[:], in_offset=None,
                        bounds_check=XR_ROWS - 1, oob_is_err=False)

    _strip_scatter_waw(nc)
    _split_excess_waits(nc)
    return nc


_NC_CACHE = {}


def _get_nc(which, blocks=None):
    key = (which, tuple(tuple(b) for b in blocks) if blocks else None)
    if key not in _NC_CACHE:
        _NC_CACHE[key] = (_build_dense() if which == "dense"
                          else _build_sparse(blocks))
    return _NC_CACHE[key]


def _routing_masks(x, Wg, bg):
    """Host-side replica of the gate: per-expert top-2 membership mask.
    Used only to derive kernel *structure* (CAP check + block lists);
    all routing values are recomputed on device."""
    xf = np.asarray(x, dtype=np.float32).reshape(NT, C)
    logits = xf @ np.asarray(Wg, dtype=np.float32) + np.asarray(
        bg, dtype=np.float32)
    m = logits.max(axis=1, keepdims=True)
    p = np.exp(logits - m)
    p /= p.sum(axis=1, keepdims=True)
    thr = np.partition(p, E - 2, axis=1)[:, E - 2:E - 1]
    return p >= thr  # [NT, E]


def _blocks_from_masks(maskmat):
    """Per slot tile st: union over experts of the token tiles whose slot
    ranges intersect it, with +-1 tile margin. Returns (blocks, ok)."""
    blocks = [set() for _ in range(NSL)]
    ok = True
    for e in range(E):
        mask = maskmat[:, e]
        cnt = mask.reshape(NG, P).sum(1)
        base = np.concatenate([[0], np.cumsum(cnt)])[:-1]
        if cnt.sum() > CAP - 32:
            ok = False
        for st in range(NSL):
            lo, hi = P * st, P * st + P
            for g in range(NG):
                if base[g] < hi and base[g] + cnt[g] > lo:
                    blocks[st].add(g)
    outb = []
    for st in range(NSL):
        s = set()
        for g in blocks[st]:
            s.update({g - 1, g, g + 1})
        outb.append(sorted(gg for gg in s if 0 <= gg < NG))
    return outb, ok


def _prep_inputs(x, W1, b1, W2, b2, Wg, bg, sparse):
    xf = np.ascontiguousarray(np.asarray(x, dtype=np.float32).reshape(NT, C))
    xt = np.ascontiguousarray(xf.T)
    if sparse:
        xrp = np.zeros((XR_ROWS, C), BF16)
        xrp[:NT] = xf.astype(BF16)
        common = {"xt": xt, "xrb": xrp}
    else:
        common = {"xt": xt, "xtb": xt.astype(BF16)}
    in_maps = []
    for e in range(E):
        perm = [e] + [i for i in range(E) if i != e]
        m = dict(common)
        m.update({
            "w1": np.ascontiguousarray(W1[e]).astype(BF16),
            "b1c": np.ascontiguousarray(b1[e].reshape(KH, P).T),
            "w2": np.ascontiguousarray(W2[e]).astype(BF16),
            "b2r": np.ascontiguousarray(b2[e].reshape(1, C)).astype(BF16),
        })
        m["wgp"] = np.ascontiguousarray(Wg[:, perm]).astype(np.float32)
        if sparse:
            m["bgc"] = np.ascontiguousarray(
                bg[perm].reshape(E, 1)).astype(np.float32)
        else:
            m["bgp"] = np.ascontiguousarray(
                bg[perm].reshape(1, E)).astype(np.float32)
        in_maps.append(m)
    return in_maps


def run(x, W1, b1, W2, b2, Wg, bg, trace=False, tmpdir=None, force=None):
    blocks = None
    if force is None:
        maskmat = _routing_masks(x, Wg, bg)
        blocks, ok = _blocks_from_masks(maskmat)
        which = "sparse" if ok else "dense"
    else:
        which = force
        if which == "sparse":
            maskmat = _routing_masks(x, Wg, bg)
            blocks, _ = _blocks_from_masks(maskmat)
    nc = _get_nc(which, blocks)
    in_maps = _prep_inputs(x, W1, b1, W2, b2, Wg, bg, which == "sparse")
    res = bu.run_bass_kernel_spmd(nc, in_maps, list(range(E)), trace=trace,
                                  tmpdir=tmpdir)
    acc = res.results[0]["out"][:NT].astype(np.float32)
    for e in range(1, E):
        acc += res.results[e]["out"][:NT]
    return acc.reshape(B, T, C), res


def kernel(x, W1, b1, W2, b2, Wg, bg):
    out, _ = run(x, W1, b1, W2, b2, Wg, bg)
    return out


# revision 34
# speedup vs baseline: 1.0045x; 1.0045x over previous
"""MoE FFN (sparse expert-parallel) Trainium2 kernel.

Strategy: expert-parallel across 8 NeuronCores. Core e holds expert e's
FFN weights and computes, for ALL 8192 tokens: the gate (fp32, on device),
then routes its expert's <=CAP selected tokens through the FFN (bf16
matmuls, fp32 accumulate), scaling by the top-2-renormalized gate weight.
Host sums the 8 partial outputs (the "psum" combine of the unshard step).

The per-core expert selection is encoded purely in the input layout: each
core receives Wg/bg with expert columns permuted so its own expert is
column 0 — the gate math is permutation-equivariant, so column 0 of the
weight matrix is always "my expert".

v2 compaction: the slot table (slot -> token id, gate weight) is built
entirely in SBUF with one-hot permutation matmuls instead of per-token
indirect DRAM scatters (8192 8-byte HBM writes cost ~170ns each in SDMA
descriptor drain — ~1.4ms — which dominated v1). For each 128-slot tile
st, only the few token tiles g whose slot ranges can intersect it
contribute; that block list is computed host-side from the routing (like
CAP already is) as a union over all 8 experts plus safety margin, so the
NEFF structure stays expert-independent (SPMD).
"""
import sys

sys.path.insert(0, "/opt/trn_rl_repo")

import numpy as np
import ml_dtypes

import bass_rust
import concourse.bass as bass
import concourse.mybir as mybir
import concourse.bass_utils as bu
from concourse.tile import TileContext

BF16 = ml_dtypes.bfloat16

B, T, C, E, H = 4, 2048, 1024, 8, 4096
NT = B * T          # 8192 tokens
P = 128
KC = C // P         # 8 k-tiles over C
KH = H // P         # 32 k-tiles over H
NG = NT // P        # 64 token tiles
GCH = 256           # gate chunk tokens (2 tiles)
NGCH = NT // GCH    # 32 gate chunks
CAP = 2176          # slot capacity (seed-0 max expert count is 2115)
NSL = CAP // P      # 17 slot tiles
FFN_CHUNKS = [(0, 4), (4, 4), (8, 4), (12, 4), (16, 1)]  # (st0, n slot tiles)
CCH = C // 512      # 2 output column chunks
XR_ROWS = NT + P    # x rows + zero row at NT (pad slots gather/scatter there)
BIG = 9999.0        # "unselected" slot id sentinel

F32 = mybir.dt.float32
BF = mybir.dt.bfloat16
I32 = mybir.dt.int32
Relu = mybir.ActivationFunctionType.Relu
Exp = mybir.ActivationFunctionType.Exp
Ident = mybir.ActivationFunctionType.Identity
Alu = mybir.AluOpType


def _split_excess_waits(nc):
    """walrus codegen allows 1 sem-wait per instruction (2 on
    EventSemaphore). Move excess waits onto same-engine EventSemaphore
    insts placed just before (engine program order preserves semantics)."""
    for f in nc.m.functions:
        for bb in f.blocks:
            new = []
            changed = False
            for inst in bb.instructions:
                si = inst.sync_info
                cap = 2 if isinstance(inst, mybir.InstEventSemaphore) else 1
                if si is not None and len(si.on_wait) > cap:
                    waits = list(si.on_wait)
                    extra, keep = waits[:-cap], waits[-cap:]
                    for i in range(0, len(extra), 2):
                        w = mybir.InstEventSemaphore(
                            name=f"{inst.name}_presem{i}", ins=[], outs=[])
                        w.engine = inst.engine
                        w.sync_info = bass_rust.SyncInfo(
                            on_wait=extra[i:i + 2], on_update=[])
                        new.append(w)
                        changed = True
                    inst.sync_info = bass_rust.SyncInfo(
                        on_wait=keep, on_update=list(si.on_update))
                new.append(inst)
            if changed:
                bb.instructions = new


def _strip_scatter_waw(nc):
    """The 20 per-slot-tile output scatters (Pool-engine DMAs with a
    dynamic out AP) get WAW-chained by Tile at whole-tensor granularity,
    each waiting the previous one's DMA-completion semaphore. Their
    writes are row-disjoint (collisions only on the trash row, whose
    value is never read), and nothing reads `out` mid-kernel, so drop
    those DMASW waits. The kernel-tail drain still covers completion."""
    n_strip = 0
    for f in nc.m.functions:
        for bb in f.blocks:
            for inst in bb.instructions:
                if (not isinstance(inst, mybir.InstDMACopy)
                        or inst.engine != mybir.EngineType.Pool
                        or not inst.outs):
                    continue
                if getattr(inst.outs[0], "dynamic_ap_info", None) is None:
                    continue
                si = inst.sync_info
                if si is None or not si.on_wait:
                    continue
                keep = [w for w in si.on_wait
                        if not str(w.ant_name).startswith("DMASW")]
                if len(keep) != len(si.on_wait):
                    n_strip += len(si.on_wait) - len(keep)
                    inst.sync_info = bass_rust.SyncInfo(
                        on_wait=keep, on_update=list(si.on_update))
    return n_strip


def _build_dense():
    nc = bass.Bass()
    xt = nc.declare_dram_parameter("xt", [C, NT], F32, isOutput=False)
    xtb = nc.declare_dram_parameter("xtb", [C, NT], BF, isOutput=False)
    w1 = nc.declare_dram_parameter("w1", [C, H], BF, isOutput=False)
    b1c = nc.declare_dram_parameter("b1c", [P, KH], F32, isOutput=False)
    w2 = nc.declare_dram_parameter("w2", [H, C], BF, isOutput=False)
    b2r = nc.declare_dram_parameter("b2r", [1, C], BF, isOutput=False)
    wgp = nc.declare_dram_parameter("wgp", [C, E], F32, isOutput=False)
    bgp = nc.declare_dram_parameter("bgp", [1, E], F32, isOutput=False)
    out = nc.declare_dram_parameter("out", [NT, C], F32, isOutput=True)
    TOKCH = 512
    NCH = NT // TOKCH

    with TileContext(nc) as tc:
        with tc.tile_pool(name="wpool", bufs=1) as wpool, \
             tc.tile_pool(name="gpool", bufs=4) as gpool, \
             tc.tile_pool(name="xgpool", bufs=2) as xgpool, \
             tc.tile_pool(name="xbpool", bufs=1) as xbpool, \
             tc.tile_pool(name="htpool", bufs=1) as htpool, \
             tc.tile_pool(name="ypool", bufs=3) as ypool, \
             tc.tile_pool(name="psg", bufs=2, space="PSUM") as psgp, \
             tc.tile_pool(name="ps1", bufs=2, space="PSUM") as ps1p, \
             tc.tile_pool(name="ps2", bufs=2, space="PSUM") as ps2p:

            # ---- resident weights / constants
            w1_sb = []
            for k in range(KC):
                t = wpool.tile([P, H], BF, tag=f"w1k{k}")
                nc.sync.dma_start(out=t[:], in_=w1[k * P:(k + 1) * P, :])
                w1_sb.append(t)
            w2_sb = []
            for h in range(KH):
                t = wpool.tile([P, C], BF, tag=f"w2k{h}")
                nc.sync.dma_start(out=t[:], in_=w2[h * P:(h + 1) * P, :])
                w2_sb.append(t)
            wg_sb = []
            for k in range(KC):
                t = wpool.tile([P, E], F32, tag=f"wgk{k}")
                nc.sync.dma_start(out=t[:], in_=wgp[k * P:(k + 1) * P, :])
                wg_sb.append(t)
            b1c_sb = wpool.tile([P, KH], F32, tag="b1c")
            nc.sync.dma_start(out=b1c_sb[:], in_=b1c[:])
            b2r_sb = wpool.tile([1, C], BF, tag="b2r")
            nc.sync.dma_start(out=b2r_sb[:], in_=b2r[:])
            bg_sb = wpool.tile([1, E], F32, tag="bgp")
            nc.sync.dma_start(out=bg_sb[:], in_=bgp[:])
            ones_f = wpool.tile([1, P], F32, tag="ones_f")
            nc.vector.memset(ones_f[:], 1.0)
            ones_b = wpool.tile([1, P], BF, tag="ones_b")
            nc.vector.memset(ones_b[:], 1.0)
            # per-token gate weight of "my" expert, column g = token tile g
            wcol = wpool.tile([P, NG], F32, tag="wcol")

            # ---- gate phase: fp32 logits -> softmax -> top2 renorm weight
            for g in range(NG):
                xg = [xgpool.tile([P, P], F32, tag=f"xg{k}", name=f"xg{k}")
                      for k in range(KC)]
                for k in range(KC):
                    nc.sync.dma_start(
                        out=xg[k][:],
                        in_=xt[k * P:(k + 1) * P, g * P:(g + 1) * P])
                psg = psgp.tile([P, E], F32)
                for k in range(KC):
                    nc.tensor.matmul(out=psg[:], lhsT=xg[k][:], rhs=wg_sb[k][:],
                                     start=(k == 0), stop=False)
                nc.tensor.matmul(out=psg[:], lhsT=ones_f[:], rhs=bg_sb[:],
                                 start=False, stop=True)
                m = gpool.tile([P, 1], F32, tag="gm")
                nc.vector.reduce_max(out=m[:], in_=psg[:],
                                     axis=mybir.AxisListType.X)
                nm = gpool.tile([P, 1], F32, tag="gnm")
                nc.vector.tensor_scalar_mul(nm[:], m[:], -1.0)
                pexp = gpool.tile([P, E], F32, tag="gpexp")
                nc.scalar.activation(pexp[:], psg[:], Exp, bias=nm[:])
                s = gpool.tile([P, 1], F32, tag="gs")
                nc.vector.reduce_sum(out=s[:], in_=pexp[:],
                                     axis=mybir.AxisListType.X)
                rs = gpool.tile([P, 1], F32, tag="grs")
                nc.vector.reciprocal(rs[:], s[:])
                pn = gpool.tile([P, E], F32, tag="gpn")
                nc.vector.tensor_scalar_mul(pn[:], pexp[:], rs[:])
                top8 = gpool.tile([P, E], F32, tag="gtop8")
                nc.vector.max(out=top8[:], in_=pn[:])
                etop = gpool.tile([P, 2], F32, tag="getop")
                nc.scalar.activation(etop[:], top8[:, 0:2], Exp)
                d = gpool.tile([P, 1], F32, tag="gd")
                nc.vector.reduce_sum(out=d[:], in_=etop[:],
                                     axis=mybir.AxisListType.X)
                rd = gpool.tile([P, 1], F32, tag="grd")
                nc.vector.reciprocal(rd[:], d[:])
                ep0 = gpool.tile([P, 1], F32, tag="gep0")
                nc.scalar.activation(ep0[:], pn[:, 0:1], Exp)
                mask0 = gpool.tile([P, 1], F32, tag="gmask0")
                nc.vector.tensor_tensor(out=mask0[:], in0=pn[:, 0:1],
                                        in1=top8[:, 1:2],
                                        op=Alu.is_ge)
                t1 = gpool.tile([P, 1], F32, tag="gt1")
                nc.vector.tensor_tensor(out=t1[:], in0=ep0[:], in1=mask0[:],
                                        op=Alu.mult)
                nc.vector.tensor_tensor(out=wcol[:, g:g + 1], in0=t1[:],
                                        in1=rd[:], op=Alu.mult)

            # ---- FFN phase
            for q in range(NCH):
                xb = [xbpool.tile([P, TOKCH], BF, tag=f"xb{k}", name=f"xb{k}")
                      for k in range(KC)]
                for k in range(KC):
                    nc.sync.dma_start(
                        out=xb[k][:],
                        in_=xtb[k * P:(k + 1) * P,
                                q * TOKCH:(q + 1) * TOKCH])
                ht = []
                for h in range(KH):
                    ps1 = ps1p.tile([P, TOKCH], F32)
                    for k in range(KC):
                        nc.tensor.matmul(
                            out=ps1[:],
                            lhsT=w1_sb[k][:, h * P:(h + 1) * P],
                            rhs=xb[k][:],
                            start=(k == 0), stop=(k == KC - 1))
                    htt = htpool.tile([P, TOKCH], BF, tag=f"ht{h}")
                    nc.scalar.activation(htt[:], ps1[:], Relu,
                                         bias=b1c_sb[:, h:h + 1])
                    ht.append(htt)
                for tt in range(TOKCH // P):
                    g = q * (TOKCH // P) + tt
                    for cc in range(CCH):
                        ps2 = ps2p.tile([P, 512], F32)
                        for h in range(KH):
                            nc.tensor.matmul(
                                out=ps2[:],
                                lhsT=ht[h][:, tt * P:(tt + 1) * P],
                                rhs=w2_sb[h][:, cc * 512:(cc + 1) * 512],
                                start=(h == 0), stop=False)
                        nc.tensor.matmul(
                            out=ps2[:], lhsT=ones_b[:],
                            rhs=b2r_sb[:, cc * 512:(cc + 1) * 512],
                            start=False, stop=True)
                        y = ypool.tile([P, 512], F32, tag="y")
                        nc.vector.tensor_scalar_mul(y[:], ps2[:],
                                                    wcol[:, g:g + 1])
                        nc.sync.dma_start(
                            out=out[g * P:(g + 1) * P,
                                    cc * 512:(cc + 1) * 512],
                            in_=y[:])

    _split_excess_waits(nc)
    return nc


def _build_sparse(blocks):
    """Sparse expert-parallel kernel. `blocks[st]` lists the token tiles
    that may contribute slots to slot tile st (host-computed union over
    experts, with margin)."""
    nc = bass.Bass()
    xt = nc.declare_dram_parameter("xt", [C, NT], F32, isOutput=False)
    xrb = nc.declare_dram_parameter("xrb", [XR_ROWS, C], BF, isOutput=False)
    w1 = nc.declare_dram_parameter("w1", [C, H], BF, isOutput=False)
    b1c = nc.declare_dram_parameter("b1c", [P, KH], F32, isOutput=False)
    w2 = nc.declare_dram_parameter("w2", [H, C], BF, isOutput=False)
    b2r = nc.declare_dram_parameter("b2r", [1, C], BF, isOutput=False)
    wgp = nc.declare_dram_parameter("wgp", [C, E], F32, isOutput=False)
    bgc = nc.declare_dram_parameter("bgc", [E, 1], F32, isOutput=False)
    out = nc.declare_dram_parameter("out", [XR_ROWS, C], F32, isOutput=True)

    with TileContext(nc) as tc:
        with tc.tile_pool(name="wpool", bufs=1) as wpool, \
             tc.tile_pool(name="gpool", bufs=2) as gpool, \
             tc.tile_pool(name="xgpool", bufs=1) as xgpool, \
             tc.tile_pool(name="ptpool", bufs=3) as ptpool, \
             tc.tile_pool(name="selpool", bufs=1) as selpool, \
             tc.tile_pool(name="xrpool", bufs=2) as xrpool, \
             tc.tile_pool(name="xtspool", bufs=1) as xtspool, \
             tc.tile_pool(name="htpool", bufs=1) as htpool, \
             tc.tile_pool(name="ypool", bufs=2) as ypool, \
             tc.tile_pool(name="psgp", bufs=2, space="PSUM") as psgp, \
             tc.tile_pool(name="pcp", bufs=2, space="PSUM") as pcp, \
             tc.tile_pool(name="ps1", bufs=2, space="PSUM") as ps1p, \
             tc.tile_pool(name="ps2", bufs=2, space="PSUM") as ps2p:

            # ---- gate weights / constants (FFN weights load later so the
            # gate's x streaming isn't queued behind 17MB of W1/W2)
            wg_sb = []
            for k in range(KC):
                t = wpool.tile([P, E], F32, tag=f"wgk{k}", name=f"wgk{k}")
                nc.sync.dma_start(out=t[:], in_=wgp[k * P:(k + 1) * P, :])
                wg_sb.append(t)
            bgc_sb = wpool.tile([E, 1], F32, tag="bgc")
            nc.sync.dma_start(out=bgc_sb[:], in_=bgc[:])
            ones_f = wpool.tile([1, P], F32, tag="ones_f")
            nc.vector.memset(ones_f[:], 1.0)
            ones_c = wpool.tile([P, 1], F32, tag="ones_c")
            nc.vector.memset(ones_c[:], 1.0)
            ones_b = wpool.tile([1, P], BF, tag="ones_b")
            nc.vector.memset(ones_b[:], 1.0)
            iotac = wpool.tile([P, 1], F32, tag="iotac")
            nc.gpsimd.iota(iotac[:], pattern=[[0, 1]], base=0,
                           channel_multiplier=1,
                           allow_small_or_imprecise_dtypes=True)
            iota2d = wpool.tile([P, P], F32, tag="iota2d")
            nc.gpsimd.iota(iota2d[:], pattern=[[1, P]], base=0,
                           channel_multiplier=0,
                           allow_small_or_imprecise_dtypes=True)
            # uts[p, s] = 1 if s > p  (strict upper triangle)
            uts_sb = wpool.tile([P, P], F32, tag="uts")
            nc.vector.tensor_scalar(out=uts_sb[:], in0=iota2d[:],
                                    scalar1=iotac[:, 0:1], scalar2=None,
                                    op0=Alu.is_gt)
            # bf16 identity for PE transposes, f32 identity for the gate
            idn_bf = wpool.tile([P, P], BF, tag="idn_bf")
            nc.vector.tensor_scalar(out=idn_bf[:], in0=iota2d[:],
                                    scalar1=iotac[:, 0:1], scalar2=None,
                                    op0=Alu.is_equal)
            idn_f = wpool.tile([E, E], F32, tag="idn_f")
            nc.vector.tensor_scalar(out=idn_f[:], in0=iota2d[0:E, 0:E],
                                    scalar1=iotac[0:E, 0:1], scalar2=None,
                                    op0=Alu.is_equal)
            # bf16 free-dim iota for the compaction one-hot compares
            # (values 0..127 are bf16-exact)
            iota2b = wpool.tile([P, P], BF, tag="iota2b")
            nc.vector.tensor_copy(out=iota2b[:], in_=iota2d[:])
            # per-token-tile matmul rhs blocks [pcol | 128g | w | 1]
            rhs_all = wpool.tile([P, 4 * NG], BF, tag="rhs_all")
            rhs_v = rhs_all[:].rearrange("p (g c) -> p c g", c=4)
            nc.gpsimd.iota(rhs_v[:, 0, :], pattern=[[0, NG]], base=0,
                           channel_multiplier=1,
                           allow_small_or_imprecise_dtypes=True)
            nc.gpsimd.iota(rhs_v[:, 1, :], pattern=[[P, NG]], base=0,
                           channel_multiplier=0,
                           allow_small_or_imprecise_dtypes=True)
            nc.vector.memset(rhs_v[:, 3, :], 1.0)
            # mask column per token tile
            mcol = wpool.tile([P, NG], F32, tag="mcol")

            # ---- gate phase: logits in [E, tok] orientation (stationary
            # Wg is only 8 columns so the fp32 matmul streams the x chunk;
            # the [tok, E] orientation pays a huge fp32 LDWEIGHTS per token
            # tile instead). PE-transpose each 128-token tile to [tok, E]
            # for the softmax. No max-subtraction: |logits| < 3 here.
            GT = 512
            for q in range(NT // GT):
                xtc = [xgpool.tile([P, GT], F32, tag=f"xtc{k}",
                                   name=f"xtc{k}") for k in range(KC)]
                for k in range(KC):
                    nc.sync.dma_start(
                        out=xtc[k][:],
                        in_=xt[k * P:(k + 1) * P, q * GT:(q + 1) * GT])
                psgT = pcp.tile([E, GT], F32, tag="pc", name="psgT")
                for k in range(KC):
                    nc.tensor.matmul(out=psgT[:], lhsT=wg_sb[k][:],
                                     rhs=xtc[k][:],
                                     start=(k == 0), stop=(k == KC - 1))
                gT = gpool.tile([E, GT], F32, tag="gT", name="gT", bufs=1)
                nc.vector.tensor_scalar_add(gT[:], psgT[:], bgc_sb[:, 0:1])
                for i in range(GT // P):
                    g = q * (GT // P) + i
                    psg = psgp.tile([P, E], F32, tag="psg", name="psg")
                    nc.tensor.transpose(out=psg[:],
                                        in_=gT[:, i * P:(i + 1) * P],
                                        identity=idn_f[:])
                    pexp = gpool.tile([P, E], F32, tag="gpexp", name="gpexp")
                    nc.scalar.activation(pexp[:], psg[:], Exp)
                    s = gpool.tile([P, 1], F32, tag="gs", name="gs")
                    nc.vector.reduce_sum(out=s[:], in_=pexp[:],
                                         axis=mybir.AxisListType.X)
                    rs = gpool.tile([P, 1], F32, tag="grs", name="grs")
                    nc.vector.reciprocal(rs[:], s[:])
                    pn = gpool.tile([P, E], F32, tag="gpn", name="gpn")
                    nc.vector.tensor_scalar_mul(pn[:], pexp[:], rs[:])
                    top8 = gpool.tile([P, E], F32, tag="gtop8", name="gtop8")
                    nc.vector.max(out=top8[:], in_=pn[:])
                    etop = gpool.tile([P, 2], F32, tag="getop", name="getop")
                    nc.scalar.activation(etop[:], top8[:, 0:2], Exp)
                    d = gpool.tile([P, 1], F32, tag="gd", name="gd")
                    nc.vector.reduce_sum(out=d[:], in_=etop[:],
                                         axis=mybir.AxisListType.X)
                    rd = gpool.tile([P, 1], F32, tag="grd", name="grd")
                    nc.vector.reciprocal(rd[:], d[:])
                    ep0 = gpool.tile([P, 1], F32, tag="gep0", name="gep0")
                    nc.scalar.activation(ep0[:], pn[:, 0:1], Exp)
                    nc.vector.tensor_tensor(out=mcol[:, g:g + 1],
                                            in0=pn[:, 0:1],
                                            in1=top8[:, 1:2],
                                            op=Alu.is_ge)
                    # w = ep0 * mask * rd  (both ops mult -> grouping-safe)
                    nc.vector.tensor_scalar(
                        out=rhs_all[:, 4 * g + 2:4 * g + 3], in0=ep0[:],
                        scalar1=mcol[:, g:g + 1], scalar2=rd[:, 0:1],
                        op0=Alu.mult, op1=Alu.mult)

            # ---- FFN weights (streamed during the gate phase)
            w1_sb = []
            for k in range(KC):
                t = wpool.tile([P, H], BF, tag=f"w1k{k}", name=f"w1k{k}")
                nc.sync.dma_start(out=t[:], in_=w1[k * P:(k + 1) * P, :])
                w1_sb.append(t)
            w2_sb = []
            for h in range(KH):
                t = wpool.tile([P, C], BF, tag=f"w2k{h}", name=f"w2k{h}")
                nc.sync.dma_start(out=t[:], in_=w2[h * P:(h + 1) * P, :])
                w2_sb.append(t)
            b1c_sb = wpool.tile([P, KH], F32, tag="b1c")
            nc.sync.dma_start(out=b1c_sb[:], in_=b1c[:])
            b2r_sb = wpool.tile([1, C], BF, tag="b2r")
            nc.sync.dma_start(out=b2r_sb[:], in_=b2r[:])

            # ---- slot positions: pos[p, g] = prefix count + tile base
            ps_cnt = pcp.tile([NG, 1], F32, tag="pc", name="ps_cnt")
            nc.tensor.matmul(out=ps_cnt[:], lhsT=mcol[:], rhs=ones_c[:],
                             start=True, stop=True)
            cnt_sb = gpool.tile([NG, 1], F32, tag="cnt_sb")
            nc.vector.tensor_copy(out=cnt_sb[:], in_=ps_cnt[:])
            ps_brow = pcp.tile([1, NG], F32, tag="pc", name="ps_brow")
            nc.tensor.matmul(out=ps_brow[:], lhsT=cnt_sb[:],
                             rhs=uts_sb[0:NG, 0:NG], start=True, stop=True)
            brow_sb = gpool.tile([1, NG], F32, tag="brow_sb")
            nc.vector.tensor_copy(out=brow_sb[:], in_=ps_brow[:])
            ps_pos = pcp.tile([P, NG], F32, tag="pc", name="ps_pos")
            nc.tensor.matmul(out=ps_pos[:], lhsT=uts_sb[:], rhs=mcol[:],
                             start=True, stop=False)
            nc.tensor.matmul(out=ps_pos[:], lhsT=ones_f[:], rhs=brow_sb[:],
                             start=False, stop=True)
            # posm = M * (pos - BIG) + BIG  (unselected -> BIG)
            pos_t = gpool.tile([P, NG], F32, tag="pos_t", bufs=1)
            nc.vector.tensor_scalar_add(pos_t[:], ps_pos[:], -BIG)
            pos_m = gpool.tile([P, NG], F32, tag="pos_m", bufs=1)
            nc.vector.tensor_tensor(out=pos_m[:], in0=pos_t[:], in1=mcol[:],
                                    op=Alu.mult)
            posm = gpool.tile([P, NG], F32, tag="posm", bufs=1)
            nc.vector.tensor_scalar_add(posm[:], pos_m[:], BIG)

            # ---- compaction: per slot tile, one-hot matmuls gather the
            # (token id, weight) pairs directly into SBUF slot order.
            id_tiles, w_tiles = {}, {}
            for st in range(NSL):
                blks = blocks[st]
                pc = pcp.tile([P, 4], F32, tag="pc", name=f"pc{st}")
                for bi, g in enumerate(blks):
                    # pt[t, s] = (s + 128*st == posm[t]), a local one-hot
                    # built in ONE fused DVE op: (iota add 128st) is_eq posm
                    pt = ptpool.tile([P, P], BF, tag="pt", name="pt")
                    nc.vector.tensor_scalar(
                        out=pt[:], in0=iota2b[:],
                        scalar1=float(P * st), scalar2=posm[:, g:g + 1],
                        op0=Alu.add, op1=Alu.is_equal)
                    nc.tensor.matmul(
                        out=pc[:], lhsT=pt[:],
                        rhs=rhs_all[:, 4 * g:4 * g + 4],
                        start=(bi == 0), stop=(bi == len(blks) - 1))
                pcsb = gpool.tile([P, 4], F32, tag="pcsb", name="pcsb")
                nc.vector.tensor_copy(out=pcsb[:], in_=pc[:])
                u = gpool.tile([P, 1], F32, tag="selu", name="selu")
                nc.vector.tensor_tensor(out=u[:], in0=pcsb[:, 0:1],
                                        in1=pcsb[:, 1:2], op=Alu.add)
                t8 = gpool.tile([P, 1], F32, tag="selt8", name="selt8")
                nc.vector.tensor_scalar_add(t8[:], u[:], float(NT))
                # idf = id + NT*(1-valid): pad slots -> trash row NT
                idf = gpool.tile([P, 1], F32, tag="selidf", name="selidf")
                nc.scalar.activation(idf[:], pcsb[:, 3:4], Ident,
                                     bias=t8[:, 0:1], scale=float(-NT))
                idi = selpool.tile([P, 1], I32, tag=f"idi{st}",
                                   name=f"idi{st}")
                nc.vector.tensor_copy(out=idi[:], in_=idf[:])
                wst = selpool.tile([P, 1], F32, tag=f"wst{st}",
                                   name=f"wst{st}")
                nc.vector.tensor_copy(out=wst[:], in_=pcsb[:, 2:3])
                id_tiles[st] = idi
                w_tiles[st] = wst

            # ---- FFN over CAP slots
            for st0, nst in FFN_CHUNKS:
                TOK = nst * P
                xts = [xtspool.tile([P, 512], BF, tag=f"xts{k}",
                                    name=f"xts{k}") for k in range(KC)]
                for i in range(nst):
                    st = st0 + i
                    xrow = xrpool.tile([P, C], BF, tag="xrow", name="xrow")
                    nc.gpsimd.indirect_dma_start(
                        out=xrow[:], out_offset=None, in_=xrb[:],
                        in_offset=bass.IndirectOffsetOnAxis(
                            ap=id_tiles[st][:, :1], axis=0),
                        bounds_check=XR_ROWS - 1, oob_is_err=False)
                    for ck in range(KC):
                        pstt = psgp.tile([P, P], BF, tag="psg", name="pstt")
                        nc.tensor.transpose(
                            out=pstt[:], in_=xrow[:, ck * P:(ck + 1) * P],
                            identity=idn_bf[:])
                        nc.vector.tensor_copy(
                            out=xts[ck][:, i * P:(i + 1) * P], in_=pstt[:])
                ht = []
                for h in range(KH):
                    ps1 = ps1p.tile([P, 512], F32, tag="ps1", name="ps1")
                    for k in range(KC):
                        nc.tensor.matmul(
                            out=ps1[:, 0:TOK],
                            lhsT=w1_sb[k][:, h * P:(h + 1) * P],
                            rhs=xts[k][:, 0:TOK],
                            start=(k == 0), stop=(k == KC - 1))
                    htt = htpool.tile([P, 512], BF, tag=f"ht{h}",
                                      name=f"ht{h}")
                    nc.scalar.activation(htt[:, 0:TOK], ps1[:, 0:TOK], Relu,
                                         bias=b1c_sb[:, h:h + 1])
                    ht.append(htt)
                for i in range(nst):
                    st = st0 + i
                    y = ypool.tile([P, C], F32, tag="y", name="y")
                    for cc in range(CCH):
                        ps2 = ps2p.tile([P, 512], F32, tag="ps2", name="ps2")
                        for h in range(KH):
                            nc.tensor.matmul(
                                out=ps2[:],
                                lhsT=ht[h][:, i * P:(i + 1) * P],
                                rhs=w2_sb[h][:, cc * 512:(cc + 1) * 512],
                                start=(h == 0), stop=False)
                        nc.tensor.matmul(
                            out=ps2[:], lhsT=ones_b[:],
                            rhs=b2r_sb[:, cc * 512:(cc + 1) * 512],
                            start=False, stop=True)
                        nc.vector.tensor_scalar_mul(
                            y[:, cc * 512:(cc + 1) * 512], ps2[:],
                            w_tiles[st][:])
                    nc.gpsimd.indirect_dma_start(
                        out=out[:],
                        out_offset=bass.IndirectOffsetOnAxis(
                            ap=id_tiles[st][:, :1], axis=0),
                        in_=y[:], in_offset=None,
                        bounds_check=XR_ROWS - 1, oob_is_err=False)

    _strip_scatter_waw(nc)
    _split_excess_waits(nc)
    return nc


_NC_CACHE = {}


def _get_nc(which, blocks=None):
    key = (which, tuple(tuple(b) for b in blocks) if blocks else None)
    if key not in _NC_CACHE:
        _NC_CACHE[key] = (_build_dense() if which == "dense"
                          else _build_sparse(blocks))
    return _NC_CACHE[key]


def _routing_masks(x, Wg, bg):
    """Host-side replica of the gate: per-expert top-2 membership mask.
    Used only to derive kernel *structure* (CAP check + block lists);
    all routing values are recomputed on device."""
    xf = np.asarray(x, dtype=np.float32).reshape(NT, C)
    logits = xf @ np.asarray(Wg, dtype=np.float32) + np.asarray(
        bg, dtype=np.float32)
    m = logits.max(axis=1, keepdims=True)
    p = np.exp(logits - m)
    p /= p.sum(axis=1, keepdims=True)
    thr = np.partition(p, E - 2, axis=1)[:, E - 2:E - 1]
    return p >= thr  # [NT, E]


def _blocks_from_masks(maskmat):
    """Per slot tile st: union over experts of the token tiles whose slot
    ranges intersect it, with +-1 tile margin. Returns (blocks, ok)."""
    blocks = [set() for _ in range(NSL)]
    ok = True
    for e in range(E):
        mask = maskmat[:, e]
        cnt = mask.reshape(NG, P).sum(1)
        base = np.concatenate([[0], np.cumsum(cnt)])[:-1]
        if cnt.sum() > CAP - 32:
            ok = False
        for st in range(NSL):
            lo, hi = P * st, P * st + P
            for g in range(NG):
                if base[g] < hi and base[g] + cnt[g] > lo:
                    blocks[st].add(g)
    outb = []
    for st in range(NSL):
        s = set()
        for g in blocks[st]:
            s.update({g - 1, g, g + 1})
        outb.append(sorted(gg for gg in s if 0 <= gg < NG))
    return outb, ok


def _prep_inputs(x, W1, b1, W2, b2, Wg, bg, sparse):
    xf = np.ascontiguousarray(np.asarray(x, dtype=np.float32).reshape(NT, C))
    xt = np.ascontiguousarray(xf.T)
    if sparse:
        xrp = np.zeros((XR_ROWS, C), BF16)
        xrp[:NT] = xf.astype(BF16)
        common = {"xt": xt, "xrb": xrp}
    else:
        common = {"xt": xt, "xtb": xt.astype(BF16)}
    in_maps = []
    for e in range(E):
        perm = [e] + [i for i in range(E) if i != e]
        m = dict(common)
        m.update({
            "w1": np.ascontiguousarray(W1[e]).astype(BF16),
            "b1c": np.ascontiguousarray(b1[e].reshape(KH, P).T),
            "w2": np.ascontiguousarray(W2[e]).astype(BF16),
            "b2r": np.ascontiguousarray(b2[e].reshape(1, C)).astype(BF16),
        })
        m["wgp"] = np.ascontiguousarray(Wg[:, perm]).astype(np.float32)
        if sparse:
            m["bgc"] = np.ascontiguousarray(
                bg[perm].reshape(E, 1)).astype(np.float32)
        else:
            m["bgp"] = np.ascontiguousarray(
                bg[perm].reshape(1, E)).astype(np.float32)
        in_maps.append(m)
    return in_maps


def run(x, W1, b1, W2, b2, Wg, bg, trace=False, tmpdir=None, force=None):
    blocks = None
    if force is None:
        maskmat = _routing_masks(x, Wg, bg)
        blocks, ok = _blocks_from_masks(maskmat)
        which = "sparse" if ok else "dense"
    else:
        which = force
        if which == "sparse":
            maskmat = _routing_masks(x, Wg, bg)
            blocks, _ = _blocks_from_masks(maskmat)
    nc = _get_nc(which, blocks)
    in_maps = _prep_inputs(x, W1, b1, W2, b2, Wg, bg, which == "sparse")
    res = bu.run_bass_kernel_spmd(nc, in_maps, list(range(E)), trace=trace,
                                  tmpdir=tmpdir)
    acc = res.results[0]["out"][:NT].astype(np.float32)
    for e in range(1, E):
        acc += res.results[e]["out"][:NT]
    return acc.reshape(B, T, C), res


def kernel(x, W1, b1, W2, b2, Wg, bg):
    out, _ = run(x, W1, b1, W2, b2, Wg, bg)
    return out
